# revision 3
# baseline (speedup 1.0000x reference)
"""Multi-head attention (B=4, S=2048, D=1024, H=16, causal) on 8 Trainium2
NeuronCores via Bass/Tile.

Three SPMD launches:
  L1  QKV projections, row-sharded: core c computes (x @ W.T + b)^T for its
      1/8 of the B*S rows, all three projections, output in [outcol, rows]
      (transposed) layout.
  L2  Attention, head-sharded: core c handles heads {2c, 2c+1} for all
      batches.  Scores are computed transposed (ST = K @ Q^T, [k, q] layout)
      so softmax's sum runs over PSUM partitions via a ones-column appended
      to V in the AV matmul - no on-chip transposes anywhere.  Causal
      structure skips upper-triangular score blocks entirely.
  L3  Output projection, row-sharded over the B*S rows.

Host work between launches is limited to reshaping/transposing shards and
the final denominator division (softmax normalization commutes with Wo).
"""

import sys

sys.path.insert(0, "/opt/trn_rl_repo")

import numpy as np

import concourse.bacc as bacc
import concourse.tile as tile
from concourse import mybir
from concourse.bass_utils import run_bass_kernel_spmd

F32 = mybir.dt.float32
F32R = mybir.dt.float32r
EXP = mybir.ActivationFunctionType.Exp
COPY = mybir.ActivationFunctionType.Copy

B, S, D, H, DK = 4, 2048, 1024, 16, 64
NCORES = 8
HPC = H // NCORES          # heads per core (2)
RPC = B * S // NCORES      # rows per core in row-sharded launches (1024)
SCALE = 1.0 / np.sqrt(DK)  # folded into the exp activation
NEG = -1e30

_CACHE = {}


def _build_proj():
    """L1: yT = (x @ W.T + b)^T for q/k/v, row shard of 1024 rows."""
    nc = bacc.Bacc(trn_type="TRN2", target_bir_lowering=False)
    ins, outs = {}, {}
    for p in ("q", "k", "v"):
        ins[p] = (
            nc.dram_tensor(f"x{p}", [D, RPC], F32R, kind="ExternalInput"),
            nc.dram_tensor(f"w{p}", [D, D], F32R, kind="ExternalInput"),
            nc.dram_tensor(f"b{p}", [128, D // 128], F32, kind="ExternalInput"),
        )
        outs[p] = nc.dram_tensor(f"{p}t", [D, RPC], F32, kind="ExternalOutput")

    KT, OCT, RC = D // 128, D // 128, RPC // 512  # 8 k-tiles, 8 oc-tiles, 2 chunks
    with tile.TileContext(nc) as tc:
        with (
            tc.tile_pool(name="big", bufs=2) as big,
            tc.tile_pool(name="bias", bufs=2) as bias,
            tc.tile_pool(name="outp", bufs=3) as outp,
            tc.tile_pool(name="ps", bufs=2, space="PSUM") as psp,
        ):
            for p in ("q", "k", "v"):
                x_d, w_d, b_d = ins[p]
                xt = big.tile([128, KT, RPC], F32R, tag="xt")
                wt = big.tile([128, KT, D], F32R, tag="wt")
                bt = bias.tile([128, OCT], F32, tag="bt")
                nc.sync.dma_start(out=xt[:], in_=x_d.rearrange("(t p) r -> p t r", p=128))
                nc.sync.dma_start(out=wt[:], in_=w_d.rearrange("(t p) o -> p t o", p=128))
                nc.sync.dma_start(out=bt[:], in_=b_d[:])
                for oc in range(OCT):
                    ps = psp.tile([128, RPC], F32, tag="ps")
                    for kt in range(KT):
                        lhs = wt[:, kt, oc * 128:(oc + 1) * 128]
                        for rc in range(RC):
                            nc.tensor.matmul(
                                ps[:, rc * 512:(rc + 1) * 512],
                                lhs,
                                xt[:, kt, rc * 512:(rc + 1) * 512],
                                start=(kt == 0),
                                stop=(kt == KT - 1),
                            )
                    ob = outp.tile([128, RPC], F32, tag="ob")
                    nc.vector.tensor_scalar_add(ob[:], ps[:], bt[:, oc:oc + 1])
                    nc.sync.dma_start(
                        out=outs[p][oc * 128:(oc + 1) * 128, :], in_=ob[:]
                    )
    nc.compile()
    return nc


def _build_attn(causal):
    """L2: attention for 2 heads x 4 batches.

    qt/kt: [128, B*S] f32r - head pair stacked on partitions (h0: 0-63,
    h1: 64-127), columns b*S+s.
    vp:    [B, 128, HPC, S//128, DK+1] f32r - V with a ones column appended
           (vp[b, p, hh, t, c] = V'[b, head hh, k = t*128+p, c]).
    mask:  [128, 896] f32 - causal additive mask master tile (causal mode),
           or maskb: [B? no - general mode uses biasT [S, S]].
    u:     [B, HPC, DK+1, S] f32 - rows 0-63 unnormalized A^T, row 64 the
           softmax denominator.
    """
    nc = bacc.Bacc(trn_type="TRN2", target_bir_lowering=False)
    qt_d = nc.dram_tensor("qt", [128, B * S], F32R, kind="ExternalInput")
    kt_d = nc.dram_tensor("kt", [128, B * S], F32R, kind="ExternalInput")
    vp_d = nc.dram_tensor("vp", [B, 128, HPC, S // 128, DK + 1], F32R,
                          kind="ExternalInput")
    if causal:
        mk_d = nc.dram_tensor("mask", [128, 896], F32, kind="ExternalInput")
    else:
        mk_d = nc.dram_tensor("maskb", [S // 128, 128, S], F32,
                              kind="ExternalInput")
    u_d = nc.dram_tensor("u", [B, HPC, DK + 1, S], F32, kind="ExternalOutput")

    NJ = S // 512            # 4 q-chunks per batch
    NT = S // 128            # 16 k-tiles per batch
    with tile.TileContext(nc) as tc:
        with (
            tc.tile_pool(name="qk", bufs=1) as qk,
            tc.tile_pool(name="vpool", bufs=2) as vpool,
            tc.tile_pool(name="epool", bufs=4) as epool,
            tc.tile_pool(name="upool", bufs=2) as upool,
            tc.tile_pool(name="mpool", bufs=2) as mpool,
            tc.tile_pool(name="stp", bufs=2, space="PSUM") as stp,
            tc.tile_pool(name="otp", bufs=2, space="PSUM") as otp,
        ):
            qt = qk.tile([128, B * S], F32R, tag="qt")
            kt = qk.tile([128, B * S], F32R, tag="kt")
            nc.sync.dma_start(out=qt[:], in_=qt_d[:])
            nc.sync.dma_start(out=kt[:], in_=kt_d[:])
            if causal:
                mk = qk.tile([128, 896], F32, tag="mk")
                nc.sync.dma_start(out=mk[:], in_=mk_d[:])
            for b in range(B):
                vp = vpool.tile([128, HPC, S // 128, DK + 1], F32R, tag="vp")
                nc.sync.dma_start(out=vp[:], in_=vp_d[b])
                us = [upool.tile([DK + 1, S], F32, tag=f"us{hh}", name=f"us{hh}") for hh in range(HPC)]
                for j in range(NJ):
                    qsl = slice(b * S + j * 512, b * S + (j + 1) * 512)
                    ots = [otp.tile([DK + 1, 512], F32, tag=f"ot{hh}",
                                    name=f"ot{hh}") for hh in range(HPC)]
                    ktiles = range(4 * j + 4) if causal else range(NT)
                    last_i = (4 * j + 3) if causal else (NT - 1)
                    for i in ktiles:
                        ksl = slice(b * S + i * 128, b * S + (i + 1) * 128)
                        st = stp.tile([128, 1024], F32, tag="st")
                        nc.tensor.matmul(st[:, 0:512], kt[0:64, ksl],
                                         qt[0:64, qsl], start=True, stop=True)
                        nc.tensor.matmul(st[:, 512:1024], kt[64:128, ksl],
                                         qt[64:128, qsl], start=True, stop=True)
                        if causal and i >= 4 * j:
                            off = 128 * i - 512 * j
                            msl = mk[:, 384 - off:896 - off]
                            nc.vector.tensor_add(st[:, 0:512], st[:, 0:512], msl)
                            nc.vector.tensor_add(st[:, 512:1024], st[:, 512:1024], msl)
                        elif not causal:
                            mb = mpool.tile([128, 512], F32, tag="mb")
                            nc.sync.dma_start(
                                out=mb[:], in_=mk_d[i, :, j * 512:(j + 1) * 512])
                            nc.vector.tensor_add(st[:, 0:512], st[:, 0:512], mb[:])
                            nc.vector.tensor_add(st[:, 512:1024], st[:, 512:1024], mb[:])
                        e = epool.tile([128, 1024], F32R, tag="e")
                        nc.scalar.activation(e[:], st[:], EXP, scale=float(SCALE))
                        for hh in range(HPC):
                            nc.tensor.matmul(
                                ots[hh][:],
                                vp[:, hh, i, :],
                                e[:, hh * 512:(hh + 1) * 512],
                                start=(i == 0),
                                stop=(i == last_i),
                            )
                    for hh in range(HPC):
                        nc.vector.tensor_copy(
                            us[hh][:, j * 512:(j + 1) * 512], ots[hh][:])
                for hh in range(HPC):
                    nc.sync.dma_start(out=u_d[b, hh], in_=us[hh][:])
    nc.compile()
    return nc


def _build_outproj():
    """L3: y = A @ Wo.T for a 1024-row shard (bias added on host)."""
    nc = bacc.Bacc(trn_type="TRN2", target_bir_lowering=False)
    at_d = nc.dram_tensor("at", [D, RPC], F32R, kind="ExternalInput")
    wo_d = nc.dram_tensor("wo", [D, D], F32R, kind="ExternalInput")
    y_d = nc.dram_tensor("y", [RPC, D], F32, kind="ExternalOutput")

    KT, RB = D // 128, RPC // 128
    with tile.TileContext(nc) as tc:
        with (
            tc.tile_pool(name="big", bufs=1) as big,
            tc.tile_pool(name="outp", bufs=3) as outp,
            tc.tile_pool(name="ps", bufs=2, space="PSUM") as psp,
        ):
            at = big.tile([128, KT, RPC], F32R, tag="at")
            wo = big.tile([128, KT, D], F32R, tag="wo")
            nc.sync.dma_start(out=at[:], in_=at_d.rearrange("(t p) r -> p t r", p=128))
            nc.sync.dma_start(out=wo[:], in_=wo_d.rearrange("(t p) o -> p t o", p=128))
            for rb in range(RB):
                ps = psp.tile([128, D], F32, tag="ps")
                for kt in range(KT):
                    lhs = at[:, kt, rb * 128:(rb + 1) * 128]
                    for oc in range(D // 512):
                        nc.tensor.matmul(
                            ps[:, oc * 512:(oc + 1) * 512],
                            lhs,
                            wo[:, kt, oc * 512:(oc + 1) * 512],
                            start=(kt == 0),
                            stop=(kt == KT - 1),
                        )
                ob = outp.tile([128, D], F32, tag="ob")
                nc.vector.tensor_copy(ob[:], ps[:])
                nc.sync.dma_start(out=y_d[rb * 128:(rb + 1) * 128, :], in_=ob[:])
    nc.compile()
    return nc


def _get(name, builder, *args):
    if name not in _CACHE:
        _CACHE[name] = builder(*args)
    return _CACHE[name]


def _causal_master_mask():
    # mm[p, c] = 0 if c >= p + 384 else NEG ; slice [384-off : 896-off]
    # masks ST tile elements where q_local (f) < off + p.
    p = np.arange(128)[:, None]
    c = np.arange(896)[None, :]
    return np.where(c >= p + 384, 0.0, NEG).astype(np.float32)


def kernel(q, k, v, mask, Wq, bq, Wk, bk, Wv, bv, Wo, bo):
    q = np.asarray(q, dtype=np.float32)
    k = np.asarray(k, dtype=np.float32)
    v = np.asarray(v, dtype=np.float32)
    mask = np.asarray(mask)
    cores = list(range(NCORES))

    # ---------------- L1: QKV projections (row-sharded) ----------------
    nc1 = _get("proj", _build_proj)
    xqT = np.ascontiguousarray(q.reshape(B * S, D).T)   # [D, B*S]
    xkT = np.ascontiguousarray(k.reshape(B * S, D).T)
    xvT = np.ascontiguousarray(v.reshape(B * S, D).T)
    wqT = np.ascontiguousarray(np.asarray(Wq, np.float32).T)
    wkT = np.ascontiguousarray(np.asarray(Wk, np.float32).T)
    wvT = np.ascontiguousarray(np.asarray(Wv, np.float32).T)
    bqt = np.ascontiguousarray(np.asarray(bq, np.float32).reshape(D // 128, 128).T)
    bkt = np.ascontiguousarray(np.asarray(bk, np.float32).reshape(D // 128, 128).T)
    bvt = np.ascontiguousarray(np.asarray(bv, np.float32).reshape(D // 128, 128).T)
    in1 = [
        {
            "xq": np.ascontiguousarray(xqT[:, c * RPC:(c + 1) * RPC]),
            "xk": np.ascontiguousarray(xkT[:, c * RPC:(c + 1) * RPC]),
            "xv": np.ascontiguousarray(xvT[:, c * RPC:(c + 1) * RPC]),
            "wq": wqT, "wk": wkT, "wv": wvT,
            "bq": bqt, "bk": bkt, "bv": bvt,
        }
        for c in cores
    ]
    r1 = run_bass_kernel_spmd(nc1, in1, core_ids=cores)
    QT = np.concatenate([r1.results[c]["qt"] for c in cores], axis=1)  # [D, B*S]
    KTm = np.concatenate([r1.results[c]["kt"] for c in cores], axis=1)
    VT = np.concatenate([r1.results[c]["vt"] for c in cores], axis=1)

    # ---------------- L2: attention (head-sharded) ----------------------
    m2 = mask.reshape(S, S)
    causal = bool(np.array_equal(m2 != 0, np.tril(np.ones((S, S), bool))))
    allones = bool((m2 != 0).all())
    nc2 = _get(("attn", causal or allones), _build_attn, causal and not allones)

    # V' per core: [B, 128, HPC, S//128, DK+1]
    Vh = VT.reshape(H, DK, B, S)                       # [h, d, b, s]
    in2 = []
    for c in cores:
        vp = np.empty((B, 128, HPC, S // 128, DK + 1), np.float32)
        for hh in range(HPC):
            h = HPC * c + hh
            # [d, b, s] -> [b, s, d] -> [b, t, p, d]
            vb = np.transpose(Vh[h], (1, 2, 0)).reshape(B, S // 128, 128, DK)
            vp[:, :, hh, :, :DK] = np.transpose(vb, (0, 2, 1, 3))
            vp[:, :, hh, :, DK] = 1.0
        m = {
            "qt": np.ascontiguousarray(QT[c * 128:(c + 1) * 128]),
            "kt": np.ascontiguousarray(KTm[c * 128:(c + 1) * 128]),
            "vp": vp,
        }
        if causal and not allones:
            m["mask"] = _causal_master_mask()
        else:
            bias = np.where(m2 != 0, 0.0, NEG).astype(np.float32)
            if allones:
                bias[:] = 0.0
            # biasT[k, q] layout, tiled [S//128, 128, S]
            m["maskb"] = np.ascontiguousarray(
                bias.T.reshape(S // 128, 128, S))
        in2.append(m)
    r2 = run_bass_kernel_spmd(nc2, in2, core_ids=cores)

    # ---------------- normalize + L3: output projection -----------------
    UA = np.empty((D, B * S), np.float32)  # A^T, normalized
    for c in cores:
        u = r2.results[c]["u"]             # [B, HPC, DK+1, S]
        for hh in range(HPC):
            h = HPC * c + hh
            a = u[:, hh, :DK, :] / u[:, hh, DK:DK + 1, :]   # [B, DK, S]
            UA[h * DK:(h + 1) * DK] = np.transpose(a, (1, 0, 2)).reshape(DK, B * S)

    nc3 = _get("outproj", _build_outproj)
    woT = np.ascontiguousarray(np.asarray(Wo, np.float32).T)
    in3 = [
        {"at": np.ascontiguousarray(UA[:, c * RPC:(c + 1) * RPC]), "wo": woT}
        for c in cores
    ]
    r3 = run_bass_kernel_spmd(nc3, in3, core_ids=cores)
    y = np.concatenate([r3.results[c]["y"] for c in cores], axis=0)
    y = y + np.asarray(bo, np.float32)[None, :]
    return y.reshape(B, S, D)


# revision 4
# speedup vs baseline: 1.4942x; 1.4942x over previous
"""Multi-head attention (B=4, S=2048, D=1024, H=16, causal) on 8 Trainium2
NeuronCores via Bass/Tile.

Three SPMD launches:
  L1  QKV projections, row-sharded: core c computes (x @ W.T + b)^T for its
      1/8 of the B*S rows, all three projections, output in [outcol, rows]
      (transposed) layout, bf16.
  L2  Attention, head-sharded: core c handles heads {2c, 2c+1} for all
      batches.  Scores are computed transposed (ST = K @ Q^T, [k, q] layout)
      so the softmax sum runs over PSUM partitions via a ones-column appended
      to V in the AV matmul - no on-chip transposes anywhere.  Causal
      structure skips upper-triangular score blocks; the triangular boundary
      is applied post-exp as a cheap 0/1 multiply on the [128,128] boundary
      strip of E plus memsets of fully-masked regions.
  L3  Output projection, row-sharded over the B*S rows.

Matmul operands are bf16 (1 cycle/row on the PE, half the DMA);
accumulation is fp32 in PSUM and the softmax denominators stay fp32.
Host work between launches is limited to reshaping/transposing shards and
the final denominator division (softmax normalization commutes with Wo).
"""

import sys

sys.path.insert(0, "/opt/trn_rl_repo")

import ml_dtypes
import numpy as np

import concourse.bacc as bacc
import concourse.tile as tile
from concourse import mybir
from concourse.bass_utils import run_bass_kernel_spmd

F32 = mybir.dt.float32
BF16 = mybir.dt.bfloat16
NPBF = ml_dtypes.bfloat16
EXP = mybir.ActivationFunctionType.Exp

B, S, D, H, DK = 4, 2048, 1024, 16, 64
NCORES = 8
HPC = H // NCORES          # heads per core (2)
RPC = B * S // NCORES      # rows per core in row-sharded launches (1024)
SCALE = 1.0 / np.sqrt(DK)  # folded into the exp activation
NEG = -1e30

_CACHE = {}


def _build_proj():
    """L1: yT = (x @ W.T + b)^T for q/k/v, row shard of 1024 rows."""
    nc = bacc.Bacc(trn_type="TRN2", target_bir_lowering=False)
    ins, outs = {}, {}
    for p in ("q", "k", "v"):
        ins[p] = (
            nc.dram_tensor(f"x{p}", [D, RPC], BF16, kind="ExternalInput"),
            nc.dram_tensor(f"w{p}", [D, D], BF16, kind="ExternalInput"),
            nc.dram_tensor(f"b{p}", [128, D // 128], F32, kind="ExternalInput"),
        )
        outs[p] = nc.dram_tensor(f"{p}t", [D, RPC], BF16, kind="ExternalOutput")

    KT, OCT, RC = D // 128, D // 128, RPC // 512  # 8 k-tiles, 8 oc-tiles, 2 chunks
    with tile.TileContext(nc) as tc:
        with (
            tc.tile_pool(name="big", bufs=2) as big,
            tc.tile_pool(name="bias", bufs=2) as bias,
            tc.tile_pool(name="outp", bufs=3) as outp,
            tc.tile_pool(name="ps", bufs=2, space="PSUM") as psp,
        ):
            for p in ("q", "k", "v"):
                x_d, w_d, b_d = ins[p]
                xt = big.tile([128, KT, RPC], BF16, tag="xt")
                wt = big.tile([128, KT, D], BF16, tag="wt")
                bt = bias.tile([128, OCT], F32, tag="bt")
                nc.sync.dma_start(out=xt[:], in_=x_d.rearrange("(t p) r -> p t r", p=128))
                nc.sync.dma_start(out=wt[:], in_=w_d.rearrange("(t p) o -> p t o", p=128))
                nc.sync.dma_start(out=bt[:], in_=b_d[:])
                for oc in range(OCT):
                    ps = psp.tile([128, RPC], F32, tag="ps")
                    for kt in range(KT):
                        lhs = wt[:, kt, oc * 128:(oc + 1) * 128]
                        for rc in range(RC):
                            nc.tensor.matmul(
                                ps[:, rc * 512:(rc + 1) * 512],
                                lhs,
                                xt[:, kt, rc * 512:(rc + 1) * 512],
                                start=(kt == 0),
                                stop=(kt == KT - 1),
                            )
                    ob = outp.tile([128, RPC], BF16, tag="ob")
                    nc.vector.tensor_scalar_add(ob[:], ps[:], bt[:, oc:oc + 1])
                    nc.sync.dma_start(
                        out=outs[p][oc * 128:(oc + 1) * 128, :], in_=ob[:]
                    )
    nc.compile()
    return nc


def _build_attn(causal):
    """L2: attention for 2 heads x 4 batches.

    qt/kt: [128, B*S] bf16 - head pair stacked on partitions (h0: 0-63,
    h1: 64-127), columns b*S+s.
    vp:    [B, 128, HPC, S//128, DK+1] bf16 - V with a ones column appended
           (vp[b, p, hh, t, c] = V'[b, head hh, k = t*128+p, c]).
    m01:   [128, 128] bf16 - causal 0/1 boundary strip (causal mode);
    maskb: [S//128, 128, S] f32 - additive bias in [k, q] layout (general).
    u:     [B, HPC, DK+1, S] f32 - rows 0-63 unnormalized A^T, row 64 the
           softmax denominator.
    """
    nc = bacc.Bacc(trn_type="TRN2", target_bir_lowering=False)
    qt_d = nc.dram_tensor("qt", [128, B * S], BF16, kind="ExternalInput")
    kt_d = nc.dram_tensor("kt", [128, B * S], BF16, kind="ExternalInput")
    vp_d = nc.dram_tensor("vp", [B, 128, HPC, S // 128, DK + 1], BF16,
                          kind="ExternalInput")
    if causal:
        mk_d = nc.dram_tensor("m01", [128, 128], BF16, kind="ExternalInput")
    else:
        mk_d = nc.dram_tensor("maskb", [S // 128, 128, S], F32,
                              kind="ExternalInput")
    u_d = nc.dram_tensor("u", [B, HPC, DK + 1, S], F32, kind="ExternalOutput")

    NJ = S // 512            # 4 q-chunks per batch
    NT = S // 128            # 16 k-tiles per batch
    with tile.TileContext(nc) as tc:
        with (
            tc.tile_pool(name="qk", bufs=1) as qk,
            tc.tile_pool(name="vpool", bufs=2) as vpool,
            tc.tile_pool(name="epool", bufs=4) as epool,
            tc.tile_pool(name="upool", bufs=2) as upool,
            tc.tile_pool(name="mpool", bufs=2) as mpool,
            tc.tile_pool(name="stp", bufs=2, space="PSUM") as stp,
            tc.tile_pool(name="otp", bufs=2, space="PSUM") as otp,
        ):
            qt = qk.tile([128, B * S], BF16, tag="qt")
            kt = qk.tile([128, B * S], BF16, tag="kt")
            nc.sync.dma_start(out=qt[:], in_=qt_d[:])
            nc.sync.dma_start(out=kt[:], in_=kt_d[:])
            if causal:
                mk = qk.tile([128, 128], BF16, tag="mk")
                nc.sync.dma_start(out=mk[:], in_=mk_d[:])
            for b in range(B):
                vp = vpool.tile([128, HPC, S // 128, DK + 1], BF16, tag="vp")
                nc.sync.dma_start(out=vp[:], in_=vp_d[b])
                us = [upool.tile([DK + 1, S], F32, tag=f"us{hh}", name=f"us{hh}")
                      for hh in range(HPC)]
                for j in range(NJ):
                    qsl = slice(b * S + j * 512, b * S + (j + 1) * 512)
                    ots = [otp.tile([DK + 1, 512], F32, tag=f"ot{hh}",
                                    name=f"ot{hh}") for hh in range(HPC)]
                    ktiles = range(4 * j + 4) if causal else range(NT)
                    last_i = (4 * j + 3) if causal else (NT - 1)
                    for i in ktiles:
                        ksl = slice(b * S + i * 128, b * S + (i + 1) * 128)
                        st = stp.tile([128, 1024], F32, tag="st")
                        nc.tensor.matmul(st[:, 0:512], kt[0:64, ksl],
                                         qt[0:64, qsl], start=True, stop=True)
                        nc.tensor.matmul(st[:, 512:1024], kt[64:128, ksl],
                                         qt[64:128, qsl], start=True, stop=True)
                        if not causal:
                            mb = mpool.tile([128, 512], F32, tag="mb")
                            nc.sync.dma_start(
                                out=mb[:], in_=mk_d[i, :, j * 512:(j + 1) * 512])
                            nc.vector.tensor_add(st[:, 0:512], st[:, 0:512], mb[:])
                            nc.vector.tensor_add(st[:, 512:1024], st[:, 512:1024],
                                                 mb[:])
                        e = epool.tile([128, 1024], BF16, tag="e")
                        nc.scalar.activation(e[:], st[:], EXP, scale=float(SCALE))
                        if causal and i >= 4 * j:
                            off = 128 * i - 512 * j
                            for hh in range(HPC):
                                o = hh * 512 + off
                                nc.vector.tensor_mul(
                                    e[:, o:o + 128], e[:, o:o + 128], mk[:])
                                if off:
                                    nc.vector.memset(
                                        e[:, hh * 512:hh * 512 + off], 0.0)
                        for hh in range(HPC):
                            nc.tensor.matmul(
                                ots[hh][:],
                                vp[:, hh, i, :],
                                e[:, hh * 512:(hh + 1) * 512],
                                start=(i == 0),
                                stop=(i == last_i),
                            )
                    for hh in range(HPC):
                        nc.vector.tensor_copy(
                            us[hh][:, j * 512:(j + 1) * 512], ots[hh][:])
                for hh in range(HPC):
                    nc.sync.dma_start(out=u_d[b, hh], in_=us[hh][:])
    nc.compile()
    return nc


def _build_outproj():
    """L3: y = A @ Wo.T for a 1024-row shard (bias added on host)."""
    nc = bacc.Bacc(trn_type="TRN2", target_bir_lowering=False)
    at_d = nc.dram_tensor("at", [D, RPC], BF16, kind="ExternalInput")
    wo_d = nc.dram_tensor("wo", [D, D], BF16, kind="ExternalInput")
    y_d = nc.dram_tensor("y", [RPC, D], F32, kind="ExternalOutput")

    KT, RB = D // 128, RPC // 128
    with tile.TileContext(nc) as tc:
        with (
            tc.tile_pool(name="big", bufs=1) as big,
            tc.tile_pool(name="outp", bufs=3) as outp,
            tc.tile_pool(name="ps", bufs=2, space="PSUM") as psp,
        ):
            at = big.tile([128, KT, RPC], BF16, tag="at")
            wo = big.tile([128, KT, D], BF16, tag="wo")
            nc.sync.dma_start(out=at[:], in_=at_d.rearrange("(t p) r -> p t r", p=128))
            nc.sync.dma_start(out=wo[:], in_=wo_d.rearrange("(t p) o -> p t o", p=128))
            for rb in range(RB):
                ps = psp.tile([128, D], F32, tag="ps")
                for kt in range(KT):
                    lhs = at[:, kt, rb * 128:(rb + 1) * 128]
                    for oc in range(D // 512):
                        nc.tensor.matmul(
                            ps[:, oc * 512:(oc + 1) * 512],
                            lhs,
                            wo[:, kt, oc * 512:(oc + 1) * 512],
                            start=(kt == 0),
                            stop=(kt == KT - 1),
                        )
                ob = outp.tile([128, D], F32, tag="ob")
                nc.vector.tensor_copy(ob[:], ps[:])
                nc.sync.dma_start(out=y_d[rb * 128:(rb + 1) * 128, :], in_=ob[:])
    nc.compile()
    return nc


def _get(name, builder, *args):
    if name not in _CACHE:
        _CACHE[name] = builder(*args)
    return _CACHE[name]


def _strip_mask01():
    # m01[p, g] = 1 where the element (k = p, q = g) of the boundary strip is
    # causally valid (g >= p), else 0.
    p = np.arange(128)[:, None]
    g = np.arange(128)[None, :]
    return (g >= p).astype(NPBF)


def kernel(q, k, v, mask, Wq, bq, Wk, bk, Wv, bv, Wo, bo):
    q = np.asarray(q, dtype=np.float32)
    k = np.asarray(k, dtype=np.float32)
    v = np.asarray(v, dtype=np.float32)
    mask = np.asarray(mask)
    cores = list(range(NCORES))

    # ---------------- L1: QKV projections (row-sharded) ----------------
    nc1 = _get("proj", _build_proj)
    xqT = np.ascontiguousarray(q.reshape(B * S, D).T.astype(NPBF))   # [D, B*S]
    xkT = np.ascontiguousarray(k.reshape(B * S, D).T.astype(NPBF))
    xvT = np.ascontiguousarray(v.reshape(B * S, D).T.astype(NPBF))
    wqT = np.ascontiguousarray(np.asarray(Wq, np.float32).T.astype(NPBF))
    wkT = np.ascontiguousarray(np.asarray(Wk, np.float32).T.astype(NPBF))
    wvT = np.ascontiguousarray(np.asarray(Wv, np.float32).T.astype(NPBF))
    bqt = np.ascontiguousarray(np.asarray(bq, np.float32).reshape(D // 128, 128).T)
    bkt = np.ascontiguousarray(np.asarray(bk, np.float32).reshape(D // 128, 128).T)
    bvt = np.ascontiguousarray(np.asarray(bv, np.float32).reshape(D // 128, 128).T)
    in1 = [
        {
            "xq": np.ascontiguousarray(xqT[:, c * RPC:(c + 1) * RPC]),
            "xk": np.ascontiguousarray(xkT[:, c * RPC:(c + 1) * RPC]),
            "xv": np.ascontiguousarray(xvT[:, c * RPC:(c + 1) * RPC]),
            "wq": wqT, "wk": wkT, "wv": wvT,
            "bq": bqt, "bk": bkt, "bv": bvt,
        }
        for c in cores
    ]
    r1 = run_bass_kernel_spmd(nc1, in1, core_ids=cores)
    QT = np.concatenate([r1.results[c]["qt"] for c in cores], axis=1)  # [D, B*S]
    KTm = np.concatenate([r1.results[c]["kt"] for c in cores], axis=1)
    VT = np.concatenate([r1.results[c]["vt"] for c in cores], axis=1)

    # ---------------- L2: attention (head-sharded) ----------------------
    m2 = mask.reshape(S, S)
    causal = bool(np.array_equal(m2 != 0, np.tril(np.ones((S, S), bool))))
    allones = bool((m2 != 0).all())
    use_causal = causal and not allones
    nc2 = _get(("attn", use_causal), _build_attn, use_causal)

    # V' per core: [B, 128, HPC, S//128, DK+1]
    Vh = VT.reshape(H, DK, B, S)                       # [h, d, b, s]
    in2 = []
    for c in cores:
        vp = np.empty((B, 128, HPC, S // 128, DK + 1), NPBF)
        for hh in range(HPC):
            h = HPC * c + hh
            # [d, b, s] -> [b, s, d] -> [b, t, p, d]
            vb = np.transpose(Vh[h], (1, 2, 0)).reshape(B, S // 128, 128, DK)
            vp[:, :, hh, :, :DK] = np.transpose(vb, (0, 2, 1, 3))
            vp[:, :, hh, :, DK] = 1.0
        m = {
            "qt": np.ascontiguousarray(QT[c * 128:(c + 1) * 128]),
            "kt": np.ascontiguousarray(KTm[c * 128:(c + 1) * 128]),
            "vp": vp,
        }
        if use_causal:
            m["m01"] = _strip_mask01()
        else:
            bias = np.where(m2 != 0, 0.0, NEG).astype(np.float32)
            if allones:
                bias[:] = 0.0
            # biasT[k, q] layout, tiled [S//128, 128, S]
            m["maskb"] = np.ascontiguousarray(bias.T.reshape(S // 128, 128, S))
        in2.append(m)
    r2 = run_bass_kernel_spmd(nc2, in2, core_ids=cores)

    # ---------------- normalize + L3: output projection -----------------
    UA = np.empty((D, B * S), np.float32)  # A^T, normalized
    for c in cores:
        u = r2.results[c]["u"]             # [B, HPC, DK+1, S]
        for hh in range(HPC):
            h = HPC * c + hh
            a = u[:, hh, :DK, :] / u[:, hh, DK:DK + 1, :]   # [B, DK, S]
            UA[h * DK:(h + 1) * DK] = np.transpose(a, (1, 0, 2)).reshape(DK, B * S)

    nc3 = _get("outproj", _build_outproj)
    UAb = UA.astype(NPBF)
    woT = np.ascontiguousarray(np.asarray(Wo, np.float32).T.astype(NPBF))
    in3 = [
        {"at": np.ascontiguousarray(UAb[:, c * RPC:(c + 1) * RPC]), "wo": woT}
        for c in cores
    ]
    r3 = run_bass_kernel_spmd(nc3, in3, core_ids=cores)
    y = np.concatenate([r3.results[c]["y"] for c in cores], axis=0)
    y = y + np.asarray(bo, np.float32)[None, :]
    return y.reshape(B, S, D)


# revision 5
# speedup vs baseline: 1.6388x; 1.0968x over previous
"""Multi-head attention (B=4, S=2048, D=1024, H=16, causal) on 8 Trainium2
NeuronCores via Bass/Tile.

Three SPMD launches:
  L1  QKV projections, row-sharded: core c computes (x @ W.T + b)^T for its
      1/8 of the B*S rows, all three projections, output in [outcol, rows]
      (transposed) layout, bf16.
  L2  Attention, head-sharded: core c handles heads {2c, 2c+1} for all
      batches.  Scores are computed transposed (ST = K @ Q^T, [k, q] layout)
      so the softmax sum runs over PSUM partitions via a ones-column appended
      to V in the AV matmul - no on-chip transposes anywhere.  Causal
      structure skips upper-triangular score blocks; the triangular boundary
      is applied post-exp as a cheap 0/1 multiply on the [128,128] boundary
      strip of E plus memsets of fully-masked regions.
  L3  Output projection, row-sharded over the B*S rows.

Matmul operands are bf16 (1 cycle/row on the PE, half the DMA);
accumulation is fp32 in PSUM and the softmax denominators stay fp32.
Host work between launches is limited to reshaping/transposing shards and
the final denominator division (softmax normalization commutes with Wo).
"""

import sys

sys.path.insert(0, "/opt/trn_rl_repo")

import ml_dtypes
import numpy as np

import concourse.bacc as bacc
import concourse.tile as tile
from concourse import mybir
from concourse.bass_utils import run_bass_kernel_spmd

F32 = mybir.dt.float32
BF16 = mybir.dt.bfloat16
NPBF = ml_dtypes.bfloat16
EXP = mybir.ActivationFunctionType.Exp

B, S, D, H, DK = 4, 2048, 1024, 16, 64
NCORES = 8
HPC = H // NCORES          # heads per core (2)
RPC = B * S // NCORES      # rows per core in row-sharded launches (1024)
SCALE = 1.0 / np.sqrt(DK)  # folded into the exp activation
NEG = -1e30

_CACHE = {}


def _build_proj():
    """L1: yT = (x @ W.T + b)^T for q/k/v, row shard of 1024 rows."""
    nc = bacc.Bacc(trn_type="TRN2", target_bir_lowering=False)
    ins, outs = {}, {}
    for p in ("q", "k", "v"):
        ins[p] = (
            nc.dram_tensor(f"x{p}", [D, RPC], BF16, kind="ExternalInput"),
            nc.dram_tensor(f"w{p}", [D, D], BF16, kind="ExternalInput"),
            nc.dram_tensor(f"b{p}", [128, D // 128], F32, kind="ExternalInput"),
        )
        outs[p] = nc.dram_tensor(f"{p}t", [D, RPC], BF16, kind="ExternalOutput")

    KT, OCT, RC = D // 128, D // 128, RPC // 512  # 8 k-tiles, 8 oc-tiles, 2 chunks
    with tile.TileContext(nc) as tc:
        with (
            tc.tile_pool(name="big", bufs=2) as big,
            tc.tile_pool(name="bias", bufs=2) as bias,
            tc.tile_pool(name="outp", bufs=3) as outp,
            tc.tile_pool(name="ps", bufs=2, space="PSUM") as psp,
        ):
            for p in ("q", "k", "v"):
                x_d, w_d, b_d = ins[p]
                xt = big.tile([128, KT, RPC], BF16, tag="xt")
                wt = big.tile([128, KT, D], BF16, tag="wt")
                bt = bias.tile([128, OCT], F32, tag="bt")
                for kt in range(KT):
                    nc.sync.dma_start(out=xt[:, kt, :],
                                      in_=x_d[kt * 128:(kt + 1) * 128, :])
                    nc.sync.dma_start(out=wt[:, kt, :],
                                      in_=w_d[kt * 128:(kt + 1) * 128, :])
                nc.sync.dma_start(out=bt[:], in_=b_d[:])
                for oc in range(OCT):
                    ps = psp.tile([128, RPC], F32, tag="ps")
                    for kt in range(KT):
                        lhs = wt[:, kt, oc * 128:(oc + 1) * 128]
                        for rc in range(RC):
                            nc.tensor.matmul(
                                ps[:, rc * 512:(rc + 1) * 512],
                                lhs,
                                xt[:, kt, rc * 512:(rc + 1) * 512],
                                start=(kt == 0),
                                stop=(kt == KT - 1),
                            )
                    ob = outp.tile([128, RPC], BF16, tag="ob")
                    nc.vector.tensor_scalar_add(ob[:], ps[:], bt[:, oc:oc + 1])
                    nc.sync.dma_start(
                        out=outs[p][oc * 128:(oc + 1) * 128, :], in_=ob[:]
                    )
    nc.compile()
    return nc


def _build_attn(causal):
    """L2: attention for 2 heads x 4 batches.

    qt/kt: [128, B*S] bf16 - head pair stacked on partitions (h0: 0-63,
    h1: 64-127), columns b*S+s.
    vp:    [B, 128, HPC, S//128, DK+1] bf16 - V with a ones column appended
           (vp[b, p, hh, t, c] = V'[b, head hh, k = t*128+p, c]).
    m01:   [128, 128] bf16 - causal 0/1 boundary strip (causal mode);
    maskb: [S//128, 128, S] f32 - additive bias in [k, q] layout (general).
    u:     [B, HPC, DK+1, S] f32 - rows 0-63 unnormalized A^T, row 64 the
           softmax denominator.
    """
    nc = bacc.Bacc(trn_type="TRN2", target_bir_lowering=False)
    qt_d = nc.dram_tensor("qt", [128, B * S], BF16, kind="ExternalInput")
    kt_d = nc.dram_tensor("kt", [128, B * S], BF16, kind="ExternalInput")
    vp_d = nc.dram_tensor("vp", [B, 128, HPC, S // 128, DK + 1], BF16,
                          kind="ExternalInput")
    if causal:
        mk_d = nc.dram_tensor("m01", [128, 128], BF16, kind="ExternalInput")
    else:
        mk_d = nc.dram_tensor("maskb", [S // 128, 128, S], F32,
                              kind="ExternalInput")
    u_d = nc.dram_tensor("u", [B, HPC, DK + 1, S], F32, kind="ExternalOutput")

    NJ = S // 512            # 4 q-chunks per batch
    NT = S // 128            # 16 k-tiles per batch
    with tile.TileContext(nc) as tc:
        with (
            tc.tile_pool(name="qk", bufs=1) as qk,
            tc.tile_pool(name="vpool", bufs=2) as vpool,
            tc.tile_pool(name="epool", bufs=4) as epool,
            tc.tile_pool(name="upool", bufs=2) as upool,
            tc.tile_pool(name="mpool", bufs=2) as mpool,
            tc.tile_pool(name="stp", bufs=3, space="PSUM") as stp,
            tc.tile_pool(name="otp", bufs=1, space="PSUM") as otp,
        ):
            qt = qk.tile([128, B * S], BF16, tag="qt")
            kt = qk.tile([128, B * S], BF16, tag="kt")
            nc.sync.dma_start(out=qt[:], in_=qt_d[:])
            nc.sync.dma_start(out=kt[:], in_=kt_d[:])
            if causal:
                mk = qk.tile([128, 128], BF16, tag="mk")
                nc.sync.dma_start(out=mk[:], in_=mk_d[:])
            for b in range(B):
                vp = vpool.tile([128, HPC, S // 128, DK + 1], BF16, tag="vp")
                nc.sync.dma_start(out=vp[:], in_=vp_d[b])
                us = [upool.tile([DK + 1, S], F32, tag=f"us{hh}", name=f"us{hh}")
                      for hh in range(HPC)]
                for j in range(NJ):
                    qsl = slice(b * S + j * 512, b * S + (j + 1) * 512)
                    ots = [otp.tile([DK + 1, 512], F32, tag=f"ot{hh}",
                                    name=f"ot{hh}") for hh in range(HPC)]
                    ktiles = range(4 * j + 4) if causal else range(NT)
                    last_i = (4 * j + 3) if causal else (NT - 1)
                    for i in ktiles:
                        ksl = slice(b * S + i * 128, b * S + (i + 1) * 128)
                        st = stp.tile([128, 1024], F32, tag="st")
                        nc.tensor.matmul(st[:, 0:512], kt[0:64, ksl],
                                         qt[0:64, qsl], start=True, stop=True)
                        nc.tensor.matmul(st[:, 512:1024], kt[64:128, ksl],
                                         qt[64:128, qsl], start=True, stop=True)
                        if not causal:
                            mb = mpool.tile([128, 512], F32, tag="mb")
                            nc.sync.dma_start(
                                out=mb[:], in_=mk_d[i, :, j * 512:(j + 1) * 512])
                            nc.vector.tensor_add(st[:, 0:512], st[:, 0:512], mb[:])
                            nc.vector.tensor_add(st[:, 512:1024], st[:, 512:1024],
                                                 mb[:])
                        e = epool.tile([128, 1024], BF16, tag="e")
                        nc.scalar.activation(e[:], st[:], EXP, scale=float(SCALE))
                        if causal and i >= 4 * j:
                            off = 128 * i - 512 * j
                            for hh in range(HPC):
                                o = hh * 512 + off
                                nc.vector.tensor_mul(
                                    e[:, o:o + 128], e[:, o:o + 128], mk[:])
                                if off:
                                    nc.vector.memset(
                                        e[:, hh * 512:hh * 512 + off], 0.0)
                        for hh in range(HPC):
                            nc.tensor.matmul(
                                ots[hh][:],
                                vp[:, hh, i, :],
                                e[:, hh * 512:(hh + 1) * 512],
                                start=(i == 0),
                                stop=(i == last_i),
                            )
                    for hh in range(HPC):
                        nc.vector.tensor_copy(
                            us[hh][:, j * 512:(j + 1) * 512], ots[hh][:])
                for hh in range(HPC):
                    nc.sync.dma_start(out=u_d[b, hh], in_=us[hh][:])
    nc.compile()
    return nc


def _build_outproj():
    """L3: y = A @ Wo.T for a 1024-row shard (bias added on host)."""
    nc = bacc.Bacc(trn_type="TRN2", target_bir_lowering=False)
    at_d = nc.dram_tensor("at", [D, RPC], BF16, kind="ExternalInput")
    wo_d = nc.dram_tensor("wo", [D, D], BF16, kind="ExternalInput")
    y_d = nc.dram_tensor("y", [RPC, D], F32, kind="ExternalOutput")

    KT, RB = D // 128, RPC // 128
    with tile.TileContext(nc) as tc:
        with (
            tc.tile_pool(name="big", bufs=1) as big,
            tc.tile_pool(name="outp", bufs=3) as outp,
            tc.tile_pool(name="ps", bufs=2, space="PSUM") as psp,
        ):
            at = big.tile([128, KT, RPC], BF16, tag="at")
            wo = big.tile([128, KT, D], BF16, tag="wo")
            for kt in range(KT):
                nc.sync.dma_start(out=at[:, kt, :],
                                  in_=at_d[kt * 128:(kt + 1) * 128, :])
                nc.sync.dma_start(out=wo[:, kt, :],
                                  in_=wo_d[kt * 128:(kt + 1) * 128, :])
            for rb in range(RB):
                ps = psp.tile([128, D], F32, tag="ps")
                for kt in range(KT):
                    lhs = at[:, kt, rb * 128:(rb + 1) * 128]
                    for oc in range(D // 512):
                        nc.tensor.matmul(
                            ps[:, oc * 512:(oc + 1) * 512],
                            lhs,
                            wo[:, kt, oc * 512:(oc + 1) * 512],
                            start=(kt == 0),
                            stop=(kt == KT - 1),
                        )
                ob = outp.tile([128, D], F32, tag="ob")
                nc.vector.tensor_copy(ob[:], ps[:])
                nc.sync.dma_start(out=y_d[rb * 128:(rb + 1) * 128, :], in_=ob[:])
    nc.compile()
    return nc


def _get(name, builder, *args):
    if name not in _CACHE:
        _CACHE[name] = builder(*args)
    return _CACHE[name]


def _strip_mask01():
    # m01[p, g] = 1 where the element (k = p, q = g) of the boundary strip is
    # causally valid (g >= p), else 0.
    p = np.arange(128)[:, None]
    g = np.arange(128)[None, :]
    return (g >= p).astype(NPBF)


def kernel(q, k, v, mask, Wq, bq, Wk, bk, Wv, bv, Wo, bo):
    q = np.asarray(q, dtype=np.float32)
    k = np.asarray(k, dtype=np.float32)
    v = np.asarray(v, dtype=np.float32)
    mask = np.asarray(mask)
    cores = list(range(NCORES))

    # ---------------- L1: QKV projections (row-sharded) ----------------
    nc1 = _get("proj", _build_proj)
    xqT = np.ascontiguousarray(q.reshape(B * S, D).T.astype(NPBF))   # [D, B*S]
    xkT = np.ascontiguousarray(k.reshape(B * S, D).T.astype(NPBF))
    xvT = np.ascontiguousarray(v.reshape(B * S, D).T.astype(NPBF))
    wqT = np.ascontiguousarray(np.asarray(Wq, np.float32).T.astype(NPBF))
    wkT = np.ascontiguousarray(np.asarray(Wk, np.float32).T.astype(NPBF))
    wvT = np.ascontiguousarray(np.asarray(Wv, np.float32).T.astype(NPBF))
    bqt = np.ascontiguousarray(np.asarray(bq, np.float32).reshape(D // 128, 128).T)
    bkt = np.ascontiguousarray(np.asarray(bk, np.float32).reshape(D // 128, 128).T)
    bvt = np.ascontiguousarray(np.asarray(bv, np.float32).reshape(D // 128, 128).T)
    in1 = [
        {
            "xq": np.ascontiguousarray(xqT[:, c * RPC:(c + 1) * RPC]),
            "xk": np.ascontiguousarray(xkT[:, c * RPC:(c + 1) * RPC]),
            "xv": np.ascontiguousarray(xvT[:, c * RPC:(c + 1) * RPC]),
            "wq": wqT, "wk": wkT, "wv": wvT,
            "bq": bqt, "bk": bkt, "bv": bvt,
        }
        for c in cores
    ]
    r1 = run_bass_kernel_spmd(nc1, in1, core_ids=cores)
    QT = np.concatenate([r1.results[c]["qt"] for c in cores], axis=1)  # [D, B*S]
    KTm = np.concatenate([r1.results[c]["kt"] for c in cores], axis=1)
    VT = np.concatenate([r1.results[c]["vt"] for c in cores], axis=1)

    # ---------------- L2: attention (head-sharded) ----------------------
    m2 = mask.reshape(S, S)
    causal = bool(np.array_equal(m2 != 0, np.tril(np.ones((S, S), bool))))
    allones = bool((m2 != 0).all())
    use_causal = causal and not allones
    nc2 = _get(("attn", use_causal), _build_attn, use_causal)

    # V' per core: [B, 128, HPC, S//128, DK+1]
    Vh = VT.reshape(H, DK, B, S)                       # [h, d, b, s]
    in2 = []
    for c in cores:
        vp = np.empty((B, 128, HPC, S // 128, DK + 1), NPBF)
        for hh in range(HPC):
            h = HPC * c + hh
            # [d, b, s] -> [b, s, d] -> [b, t, p, d]
            vb = np.transpose(Vh[h], (1, 2, 0)).reshape(B, S // 128, 128, DK)
            vp[:, :, hh, :, :DK] = np.transpose(vb, (0, 2, 1, 3))
            vp[:, :, hh, :, DK] = 1.0
        m = {
            "qt": np.ascontiguousarray(QT[c * 128:(c + 1) * 128]),
            "kt": np.ascontiguousarray(KTm[c * 128:(c + 1) * 128]),
            "vp": vp,
        }
        if use_causal:
            m["m01"] = _strip_mask01()
        else:
            bias = np.where(m2 != 0, 0.0, NEG).astype(np.float32)
            if allones:
                bias[:] = 0.0
            # biasT[k, q] layout, tiled [S//128, 128, S]
            m["maskb"] = np.ascontiguousarray(bias.T.reshape(S // 128, 128, S))
        in2.append(m)
    r2 = run_bass_kernel_spmd(nc2, in2, core_ids=cores)

    # ---------------- normalize + L3: output projection -----------------
    UA = np.empty((D, B * S), np.float32)  # A^T, normalized
    for c in cores:
        u = r2.results[c]["u"]             # [B, HPC, DK+1, S]
        for hh in range(HPC):
            h = HPC * c + hh
            a = u[:, hh, :DK, :] / u[:, hh, DK:DK + 1, :]   # [B, DK, S]
            UA[h * DK:(h + 1) * DK] = np.transpose(a, (1, 0, 2)).reshape(DK, B * S)

    nc3 = _get("outproj", _build_outproj)
    UAb = UA.astype(NPBF)
    woT = np.ascontiguousarray(np.asarray(Wo, np.float32).T.astype(NPBF))
    in3 = [
        {"at": np.ascontiguousarray(UAb[:, c * RPC:(c + 1) * RPC]), "wo": woT}
        for c in cores
    ]
    r3 = run_bass_kernel_spmd(nc3, in3, core_ids=cores)
    y = np.concatenate([r3.results[c]["y"] for c in cores], axis=0)
    y = y + np.asarray(bo, np.float32)[None, :]
    return y.reshape(B, S, D)
